# revision 6
# baseline (speedup 1.0000x reference)
"""Trainium2 Bass kernel for nn_AutoRegressive (6-layer transformer decoder).

Sharding: 4 sample-pairs x 2-way sequence split over 8 NeuronCores.
Core c handles sample c//2, query-token half c%2 (256 tokens).
Per layer, each core computes K/V for its own token half; a pairwise
AllGather assembles full-length K/V. Cross-attention K/V come from the
(constant) packed memory the same way. FFN/LN/projections are per-token.

Device layout: activations transposed [C on partitions (8 chunks of 128),
tokens on free dim]. Matmuls in bf16 with fp32 PSUM accumulation.
Softmax without max-subtraction (scores are small), multiplicative 0/1
mask after exp, denominators via ones-matmuls, normalization via
gpsimd partition_broadcast.

All weights are pre-permuted on the host into partition-major tiles
([128 partitions, free...]) so every weight DMA is a linear copy with
multi-KB per-partition lines (256B-packet DMA was the v1 bottleneck).
"""
import numpy as np
import ml_dtypes

import concourse.bass as bass
import concourse.tile as tile
from concourse import bacc, mybir
from concourse.bass_utils import run_bass_kernel_spmd

F32 = mybir.dt.float32
BF16 = mybir.dt.bfloat16
AF = mybir.ActivationFunctionType
ALU = mybir.AluOpType

B, LT, LA = 4, 64, 448
C, H, DFF, NL = 1024, 16, 4096, 6
VOCAB, CODEC = 256, 1024
L = LT + LA          # 512
DH = C // H          # 64
NCH = C // 128       # 8 feature chunks
NJC = L // 128       # 4 key chunks
NFF = DFF // 128     # 32
NVC = CODEC // 128   # 8
EPS = 1e-5

N_CORES = 8
REPLICA_GROUPS = [[0, 1], [2, 3], [4, 5], [6, 7]]


# ----------------------------------------------------------------------------
# host-side prep
# ----------------------------------------------------------------------------

def _pe_np(length, dim):
    pos = np.arange(length, dtype=np.float32)[:, None]
    div = np.exp(np.arange(0, dim, 2, dtype=np.float32) * (-np.log(10000.0) / dim))
    ang = pos * div
    out = np.zeros((length, dim), np.float32)
    out[:, 0::2] = np.sin(ang)
    out[:, 1::2] = np.cos(ang)
    return out


def host_pack_and_mask(inputs):
    text = np.asarray(inputs["text"]).astype(np.int64)
    audio = np.asarray(inputs["audio"]).astype(np.int64)
    tl = np.asarray(inputs["text_len"]).astype(np.int64)
    al = np.asarray(inputs["audio_len"]).astype(np.int64)
    text_emb = np.asarray(inputs["text_emb"], dtype=np.float32)
    audio_emb = np.asarray(inputs["audio_emb"], dtype=np.float32)

    text_e = text_emb[text] + _pe_np(LT, C)[None]      # [B, LT, C]
    audio_e = audio_emb[audio] + _pe_np(LA, C)[None]   # [B, LA, C]

    packed = np.zeros((B, L, C), np.float32)
    m01 = np.zeros((B, L, L), np.float32)
    i = np.arange(L)[:, None]
    j = np.arange(L)[None, :]
    for b in range(B):
        t, a = int(tl[b]), int(al[b])
        packed[b, :t] = text_e[b, :t]
        packed[b, t:t + a] = audio_e[b, :a]
        il = t + a
        masked = (j > i) & (j >= t) & (i < il) & (j < il)
        m01[b] = np.where(masked, 0.0, 1.0)
    return packed, m01


def _bf(x):
    return np.ascontiguousarray(x).astype(ml_dtypes.bfloat16)


def _pmajor(x, pchunks, tile_w):
    """[R=pchunks*128, W] -> tiles [W//tile_w, 128, pchunks, tile_w] p-major."""
    R, W = x.shape
    assert R == pchunks * 128 and W % tile_w == 0
    return np.ascontiguousarray(
        x.reshape(pchunks, 128, W // tile_w, tile_w).transpose(2, 1, 0, 3))


def host_prep_weights(inputs):
    """Transpose weights to lhsT/rhs conventions, fold the attention scale
    into q, cast to bf16, pre-permute into partition-major DMA tiles."""
    g = lambda n: np.asarray(inputs[n], dtype=np.float32)
    w = {}
    scale = DH ** -0.5
    for pfx in ("sa", "ca"):
        qkv_w = g(f"{pfx}_qkv_w").copy()        # [NL, 3C, C]
        qkv_b = g(f"{pfx}_qkv_b").copy()        # [NL, 3C]
        qkv_w[:, :C] *= scale
        qkv_b[:, :C] *= scale
        qkv_t = qkv_w.transpose(0, 2, 1)        # [NL, C(in), 3C(out)]
        # q/k tiles: [NL, 16, 128, 8, 128]; v: [NL, 2, 128, 8, 512]
        w[f"{pfx}_qk_t"] = _bf(np.stack(
            [_pmajor(qkv_t[l, :, :2 * C], NCH, 128) for l in range(NL)]))
        w[f"{pfx}_v_t"] = _bf(np.stack(
            [_pmajor(qkv_t[l, :, 2 * C:], NCH, 512) for l in range(NL)]))
        w[f"{pfx}_qkv_b"] = np.ascontiguousarray(qkv_b)
        o_t = g(f"{pfx}_out_w").transpose(0, 2, 1)   # [NL, C(d), C(c)]
        w[f"{pfx}_o_t"] = _bf(np.stack(
            [_pmajor(o_t[l], NCH, 128) for l in range(NL)]))  # [NL, 8, 128, 8, 128]
        w[f"{pfx}_o_b"] = np.ascontiguousarray(g(f"{pfx}_out_b"))
    w1_t = g("lin1_w").transpose(0, 2, 1)       # [NL, C, DFF]
    w["w1_t"] = _bf(np.stack([_pmajor(w1_t[l], NCH, 128) for l in range(NL)]))
    w["b1"] = np.ascontiguousarray(g("lin1_b"))
    w2_t = g("lin2_w").transpose(0, 2, 1)       # [NL, DFF, C]
    w["w2_t"] = _bf(np.stack([_pmajor(w2_t[l], NFF, 128) for l in range(NL)]))
    w["b2"] = np.ascontiguousarray(g("lin2_b"))
    ln_g = np.stack([g("ln1_g"), g("ln2_g"), g("ln3_g")], axis=1)  # [NL, 3, C]
    ln_b = np.stack([g("ln1_b"), g("ln2_b"), g("ln3_b")], axis=1)
    w["ln_g"] = np.ascontiguousarray(ln_g)
    w["ln_b"] = np.ascontiguousarray(ln_b)
    w["embt"] = _bf(_pmajor(g("audio_emb").T, NCH, 128))  # [8, 128, 8, 128]
    return w


# ----------------------------------------------------------------------------
# device program
# ----------------------------------------------------------------------------

def build_program(n_layers=NL, debug_layers=False, probe=False, trivial_ln=False):
    """SPMD program for one core: sample = pair, token half = rank in pair."""
    NI = L // 2  # 256 query tokens per core
    NJO = NI // 128  # own key chunks (2)

    nc = bacc.Bacc(None, target_bir_lowering=False)

    # --- external I/O (all partition-major)
    xt = nc.dram_tensor("xt", [128, NCH, NI], F32, kind="ExternalInput")
    memt = nc.dram_tensor("memt", [128, NCH, L], BF16, kind="ExternalInput")
    m01t = nc.dram_tensor("m01t", [128, NJC, NI], BF16, kind="ExternalInput")
    sa_qk_t = nc.dram_tensor("sa_qk_t", [n_layers, 16, 128, NCH, 128], BF16, kind="ExternalInput")
    sa_v_t = nc.dram_tensor("sa_v_t", [n_layers, 2, 128, NCH, 512], BF16, kind="ExternalInput")
    sa_qkv_b = nc.dram_tensor("sa_qkv_b", [n_layers, 3 * C], F32, kind="ExternalInput")
    sa_o_t = nc.dram_tensor("sa_o_t", [n_layers, NCH, 128, NCH, 128], BF16, kind="ExternalInput")
    sa_o_b = nc.dram_tensor("sa_o_b", [n_layers, C], F32, kind="ExternalInput")
    ca_qk_t = nc.dram_tensor("ca_qk_t", [n_layers, 16, 128, NCH, 128], BF16, kind="ExternalInput")
    ca_v_t = nc.dram_tensor("ca_v_t", [n_layers, 2, 128, NCH, 512], BF16, kind="ExternalInput")
    ca_qkv_b = nc.dram_tensor("ca_qkv_b", [n_layers, 3 * C], F32, kind="ExternalInput")
    ca_o_t = nc.dram_tensor("ca_o_t", [n_layers, NCH, 128, NCH, 128], BF16, kind="ExternalInput")
    ca_o_b = nc.dram_tensor("ca_o_b", [n_layers, C], F32, kind="ExternalInput")
    w1_t = nc.dram_tensor("w1_t", [n_layers, NFF, 128, NCH, 128], BF16, kind="ExternalInput")
    b1 = nc.dram_tensor("b1", [n_layers, DFF], F32, kind="ExternalInput")
    w2_t = nc.dram_tensor("w2_t", [n_layers, NCH, 128, NFF, 128], BF16, kind="ExternalInput")
    b2 = nc.dram_tensor("b2", [n_layers, C], F32, kind="ExternalInput")
    ln_g = nc.dram_tensor("ln_g", [n_layers, 3, C], F32, kind="ExternalInput")
    ln_b = nc.dram_tensor("ln_b", [n_layers, 3, C], F32, kind="ExternalInput")
    embt = nc.dram_tensor("embt", [NVC, 128, NCH, 128], BF16, kind="ExternalInput")
    logits_t = nc.dram_tensor("logits_t", [NVC, 128, NI], F32, kind="ExternalOutput")
    dbg = None
    if debug_layers:
        dbg = nc.dram_tensor("dbg", [n_layers, C, NI], F32, kind="ExternalOutput")
    probes = None
    if probe:
        probes = {
            "p_kt": nc.dram_tensor("p_kt", [NCH, 128, L], BF16, kind="ExternalOutput"),
            "p_vt": nc.dram_tensor("p_vt", [NJC, 128, C], BF16, kind="ExternalOutput"),
            "p_qt": nc.dram_tensor("p_qt", [NCH, 128, NI], BF16, kind="ExternalOutput"),
            "p_e": nc.dram_tensor("p_e", [NJC, 128, 8, NI], BF16, kind="ExternalOutput"),
            "p_y": nc.dram_tensor("p_y", [NCH, 128, NI], BF16, kind="ExternalOutput"),
            "p_pr": nc.dram_tensor("p_pr", [NCH, 128, NI], F32, kind="ExternalOutput"),
        }

    # --- internal dram for collectives (one pair per layer per attn kind)
    KVSZ = C * NI  # elements in each of kT_own / v_own packs
    cc_in = {}
    cc_out = {}
    for l in range(n_layers):
        for kind in ("sa", "ca"):
            cc_in[(kind, l)] = nc.dram_tensor(f"{kind}_cci_{l}", [2 * KVSZ], BF16)
            cc_out[(kind, l)] = nc.dram_tensor(f"{kind}_cco_{l}", [2, 2 * KVSZ], BF16)

    with tile.TileContext(nc) as tc:
        _build_body(nc, tc, locals(), n_layers, NI, NJO, dbg, trivial_ln)
    nc.finalize()
    return nc


def _build_body(nc, tc, T, n_layers, NI, NJO, dbg, trivial_ln=False):
    from contextlib import ExitStack
    ctx = ExitStack()
    with ctx:
        const = ctx.enter_context(tc.tile_pool(name="const", bufs=1))
        xpool = ctx.enter_context(tc.tile_pool(name="xpool", bufs=2))
        pre = ctx.enter_context(tc.tile_pool(name="pre", bufs=1))
        kvp = ctx.enter_context(tc.tile_pool(name="kvp", bufs=1))
        kvo = ctx.enter_context(tc.tile_pool(name="kvo", bufs=2))
        ep = ctx.enter_context(tc.tile_pool(name="ep", bufs=1))
        yp = ctx.enter_context(tc.tile_pool(name="yp", bufs=2))
        hp = ctx.enter_context(tc.tile_pool(name="hp", bufs=1))
        wp = ctx.enter_context(tc.tile_pool(name="wp", bufs=6))
        wv = ctx.enter_context(tc.tile_pool(name="wv", bufs=1))
        wb = ctx.enter_context(tc.tile_pool(name="wb", bufs=2))
        sm = ctx.enter_context(tc.tile_pool(name="sm", bufs=2))
        lnp = ctx.enter_context(tc.tile_pool(name="lnp", bufs=1))
        ps = ctx.enter_context(tc.tile_pool(name="ps", bufs=4, space="PSUM"))
        psd = ctx.enter_context(tc.tile_pool(name="psd", bufs=2, space="PSUM"))
        psl = ctx.enter_context(tc.tile_pool(name="psl", bufs=2, space="PSUM"))

        # ---- constants
        memt_t = const.tile([128, NCH, L], BF16)
        nc.sync.dma_start(out=memt_t, in_=T["memt"][:])
        m01_t = const.tile([128, NJC, NI], BF16)
        nc.sync.dma_start(out=m01_t, in_=T["m01t"][:])
        ones_bf = const.tile([128, 1], BF16)
        nc.vector.memset(ones_bf, 1.0)
        ones_f = const.tile([128, 1], F32)
        nc.vector.memset(ones_f, 1.0)
        eps_t = const.tile([1, 1], F32)
        nc.vector.memset(eps_t, EPS)

        # ---- layer 0 activations
        x_f = xpool.tile([128, NCH, NI], F32, tag="x")
        nc.sync.dma_start(out=x_f, in_=T["xt"][:])
        xb = xpool.tile([128, NCH, NI], BF16, tag="xb")
        nc.vector.tensor_copy(xb, x_f)

        memt_bf = memt_t  # alias

        def _layer_bias(kind, l):
            bt = sm.tile([128, 24], F32, tag=f"b_{kind}")
            nc.sync.dma_start(
                out=bt, in_=T[f"{kind}_qkv_b"][l].rearrange("(dc p) -> p dc", p=128))
            return bt

        def kv_own_and_gather(kind, l, src_bf, src_islice):
            """Compute own-half K/V, pack, AllGather; return full
            kT [128,NCH,L] bf16 and v [128,NJC,C] bf16 (global token order)."""
            qk_t = T[f"{kind}_qk_t"]
            bias_tile = _layer_bias(kind, l)
            k_own = kvo.tile([128, NCH, NI], BF16, tag="k_own")
            for dc in range(NCH):
                wk = wp.tile([128, NCH, 128], BF16, tag="wA")
                nc.sync.dma_start(out=wk, in_=qk_t[l, 8 + dc])
                acc = ps.tile([128, NI], F32, tag="mm")
                for cc in range(NCH):
                    nc.tensor.matmul(acc[:], wk[:, cc], src_bf[:, cc, src_islice],
                                     start=(cc == 0), stop=(cc == NCH - 1))
                nc.vector.tensor_scalar(k_own[:, dc], acc[:],
                                        bias_tile[:, 8 + dc:9 + dc], None, ALU.add)
            # v_own [128, NJO(j), C]  (v-bias applied post-normalize)
            v_own = kvo.tile([128, NJO, C], BF16, tag="v_own")
            for ds in range(2):
                wvt = wv.tile([128, NCH, 512], BF16, tag="wV")
                nc.sync.dma_start(out=wvt, in_=T[f"{kind}_v_t"][l, ds])
                for jo in range(NJO):
                    acc = ps.tile([128, 512], F32, tag="mm")
                    for cc in range(NCH):
                        lhs = src_bf[:, cc, src_islice][:, jo * 128:(jo + 1) * 128]
                        nc.tensor.matmul(acc[:], lhs, wvt[:, cc],
                                         start=(cc == 0), stop=(cc == NCH - 1))
                    nc.vector.tensor_copy(v_own[:, jo, ds * 512:(ds + 1) * 512], acc[:])
            # pack to dram: [kT_own | v_own], both partition-major
            cci = T["cc_in"][(kind, l)]
            cco = T["cc_out"][(kind, l)]
            KVSZ = C * NI
            nc.sync.dma_start(
                out=cci[0:KVSZ].rearrange("(p dc i) -> p dc i", p=128, i=NI),
                in_=k_own)
            nc.sync.dma_start(
                out=cci[KVSZ:2 * KVSZ].rearrange("(p jo d) -> p jo d", p=128, d=C),
                in_=v_own)
            nc.gpsimd.collective_compute(
                "AllGather", ALU.bypass, ins=[cci[:]], outs=[cco[:]],
                replica_groups=REPLICA_GROUPS)
            kt = kvp.tile([128, NCH, L], BF16, tag=f"kt_{kind}")
            vt = kvp.tile([128, NJC, C], BF16, tag=f"vt_{kind}")
            for blk in range(2):
                nc.sync.dma_start(
                    out=kt[:, :, blk * NI:(blk + 1) * NI],
                    in_=cco[blk, 0:KVSZ].rearrange("(p dc i) -> p dc i", p=128, i=NI))
                nc.sync.dma_start(
                    out=vt[:, blk * NJO:(blk + 1) * NJO, :],
                    in_=cco[blk, KVSZ:2 * KVSZ].rearrange("(p jo d) -> p jo d", p=128, d=C))
            return kt, vt

        def q_proj(kind, l, bias_tile):
            qt = yp.tile([128, NCH, NI], BF16, tag="qt")
            qk_t = T[f"{kind}_qk_t"]
            for dc in range(NCH):
                wq = wp.tile([128, NCH, 128], BF16, tag="wA")
                nc.sync.dma_start(out=wq, in_=qk_t[l, dc])
                acc = ps.tile([128, NI], F32, tag="mm")
                for cc in range(NCH):
                    nc.tensor.matmul(acc[:], wq[:, cc], xb_cur[0][:, cc],
                                     start=(cc == 0), stop=(cc == NCH - 1))
                nc.vector.tensor_scalar(qt[:, dc], acc[:],
                                        bias_tile[:, dc:dc + 1], None, ALU.add)
            return qt

        def attention(kind, l, kt, vt, qt, bias_tile, masked, probes=None):
            """-> Y bf16 [128, NCH, NI] (normalized, v-bias added)."""
            Y = yp.tile([128, NCH, NI], BF16, tag="Y")
            for g in range(2):           # head groups of 8
                et = ep.tile([128, NJC, 8, NI], BF16, tag="ee")
                if probes is not None and g == 0:
                    et_probe = et
                for hh in range(8):
                    h = g * 8 + hh
                    hc, hs = h // 2, (h % 2) * 64
                    for jc in range(NJC):
                        sc = ps.tile([128, NI], F32, tag="mm")
                        nc.tensor.matmul(
                            sc[:],
                            kt[hs:hs + 64, hc, jc * 128:(jc + 1) * 128],
                            qt[hs:hs + 64, hc, :], start=True, stop=True)
                        nc.scalar.activation(et[:, jc, hh], sc[:], AF.Exp)
                        if masked:
                            nc.vector.tensor_mul(et[:, jc, hh], et[:, jc, hh],
                                                 m01_t[:, jc])
                for hp2 in range(4):     # head pairs within group
                    dc = g * 4 + hp2
                    ypv = ps.tile([128, NI], F32, tag="mm")
                    dA = psd.tile([1, NI], F32, tag="den")
                    dB = psd.tile([1, NI], F32, tag="den")
                    for jc in range(NJC):
                        nc.tensor.matmul(
                            ypv[0:64, :], vt[:, jc, dc * 128:dc * 128 + 64],
                            et[:, jc, 2 * hp2], start=(jc == 0), stop=(jc == NJC - 1),
                            tile_position=(0, 0))
                        nc.tensor.matmul(
                            ypv[64:128, :], vt[:, jc, dc * 128 + 64:dc * 128 + 128],
                            et[:, jc, 2 * hp2 + 1], start=(jc == 0), stop=(jc == NJC - 1),
                            tile_position=(0, 64))
                        nc.tensor.matmul(dA[:], ones_bf[:], et[:, jc, 2 * hp2],
                                         start=(jc == 0), stop=(jc == NJC - 1))
                        nc.tensor.matmul(dB[:], ones_bf[:], et[:, jc, 2 * hp2 + 1],
                                         start=(jc == 0), stop=(jc == NJC - 1))
                    rec = sm.tile([1, 2, NI], F32, tag="rec")
                    nc.vector.reciprocal(rec[:, 0], dA[:])
                    nc.vector.reciprocal(rec[:, 1], dB[:])
                    recbA = sm.tile([128, NI], F32, tag="recbA")
                    recbB = sm.tile([128, NI], F32, tag="recbB")
                    nc.gpsimd.partition_broadcast(recbA[:], rec[:, 0])
                    nc.gpsimd.partition_broadcast(recbB[:], rec[:, 1])
                    tY = sm.tile([128, NI], F32, tag="tY")
                    nc.vector.tensor_mul(tY[0:64, :], ypv[0:64, :], recbA[0:64, :])
                    nc.vector.tensor_mul(tY[64:128, :], ypv[64:128, :], recbB[64:128, :])
                    nc.scalar.activation(Y[:, dc], tY[:], AF.Identity,
                                         bias=bias_tile[:, 16 + dc:17 + dc])
                if probes is not None and g == 0:
                    nc.sync.dma_start(
                        out=probes["p_e"].rearrange("jc p h i -> p jc h i"),
                        in_=et_probe)
            return Y

        def out_proj_residual(kind, l, Y):
            """attn out-proj + bias + residual -> preln f32 [128, NCH, NI]."""
            ot = T[f"{kind}_o_t"]
            obt = sm.tile([128, NCH], F32, tag="ob")
            nc.sync.dma_start(out=obt,
                              in_=T[f"{kind}_o_b"][l].rearrange("(cc p) -> p cc", p=128))
            pr = pre.tile([128, NCH, NI], F32, tag="preln")
            for cc in range(NCH):
                wo = wp.tile([128, NCH, 128], BF16, tag="wA")
                nc.sync.dma_start(out=wo, in_=ot[l, cc])
                acc = ps.tile([128, NI], F32, tag="mm")
                for dc in range(NCH):
                    nc.tensor.matmul(acc[:], wo[:, dc], Y[:, dc],
                                     start=(dc == 0), stop=(dc == NCH - 1))
                tmp = sm.tile([128, NI], F32, tag="tY")
                nc.vector.tensor_scalar(tmp[:], acc[:], obt[:, cc:cc + 1], None, ALU.add)
                nc.vector.tensor_add(pr[:, cc], tmp[:], x_cur[0][:, cc])
            return pr

        def layer_norm(l, idx, pr, lg_t, lb_t):
            """LN over partitions-dim features; returns (x_f32, xb_bf16)."""
            s_ps = psl.tile([1, NI], F32, tag="lns")
            q_ps = psl.tile([1, NI], F32, tag="lns")
            for cc in range(NCH):
                xsq = sm.tile([128, NI], F32, tag="xsq")
                nc.scalar.activation(xsq[:], pr[:, cc], AF.Square)
                nc.tensor.matmul(s_ps[:], ones_f[:], pr[:, cc],
                                 start=(cc == 0), stop=(cc == NCH - 1))
                nc.tensor.matmul(q_ps[:], ones_f[:], xsq[:],
                                 start=(cc == 0), stop=(cc == NCH - 1))
            mu = lnp.tile([1, NI], F32, tag="mu")
            nc.scalar.activation(mu[:], s_ps[:], AF.Identity, scale=1.0 / C)
            var = lnp.tile([1, NI], F32, tag="var")
            nc.scalar.activation(var[:], q_ps[:], AF.Identity, scale=1.0 / C)
            musq = lnp.tile([1, NI], F32, tag="musq")
            nc.vector.tensor_mul(musq[:], mu[:], mu[:])
            nc.vector.tensor_sub(var[:], var[:], musq[:])
            sd = lnp.tile([1, NI], F32, tag="sd")
            nc.scalar.activation(sd[:], var[:], AF.Sqrt, bias=eps_t[:])
            rmur = lnp.tile([1, 2, NI], F32, tag="rmur")
            nc.vector.reciprocal(rmur[:, 0], sd[:])
            nc.vector.tensor_mul(rmur[:, 1], mu[:], rmur[:, 0])
            rb = lnp.tile([128, 2, NI], F32, tag="rb")
            nc.gpsimd.partition_broadcast(rb[:], rmur[:])
            x_new = xpool.tile([128, NCH, NI], F32, tag="x")
            xb_new = xpool.tile([128, NCH, NI], BF16, tag="xb")
            for cc in range(NCH):
                t1 = sm.tile([128, NI], F32, tag="t1")
                nc.vector.tensor_mul(t1[:], pr[:, cc], rb[:, 0])
                if trivial_ln:
                    nc.vector.tensor_sub(x_new[:, cc], t1[:], rb[:, 1])
                else:
                    nc.vector.tensor_sub(t1[:], t1[:], rb[:, 1])
                    nc.scalar.activation(x_new[:, cc], t1[:], AF.Identity,
                                         bias=lb_t[:, idx, cc:cc + 1],
                                         scale=lg_t[:, idx, cc:cc + 1])
                nc.vector.tensor_copy(xb_new[:, cc], x_new[:, cc])
            return x_new, xb_new

        # mutable current-activation refs
        x_cur = [x_f]
        xb_cur = [xb]

        for l in range(n_layers):
            lg_t = sm.tile([128, 3, NCH], F32, tag="lg")
            lb_t = sm.tile([128, 3, NCH], F32, tag="lb")
            nc.sync.dma_start(out=lg_t,
                              in_=T["ln_g"][l].rearrange("three (cc p) -> p three cc", p=128))
            nc.sync.dma_start(out=lb_t,
                              in_=T["ln_b"][l].rearrange("three (cc p) -> p three cc", p=128))

            # SA K/V + AllGather kick first (x just became available)
            sa_bias = _layer_bias("sa", l)
            with nc.named_scope(f"L{l}_sakv"):
                kt_sa, vt_sa = kv_own_and_gather("sa", l, xb_cur[0], slice(0, NI))

            # --- self-attention
            with nc.named_scope(f"L{l}_saq"):
                qt = q_proj("sa", l, sa_bias)
            # CA K/V from constant memory (memt is pre-rolled per core: own half
            # first, so the SPMD own-slice is always cols [0:NI]; the AllGather
            # restores global order). Emitted here as a lower-priority gap
            # filler for the SA AllGather wait and the LN1 serial chain.
            ca_bias = _layer_bias("ca", l)
            with nc.named_scope(f"L{l}_cakv"):
                kt_ca, vt_ca = kv_own_and_gather("ca", l, memt_bf, slice(0, NI))

            with nc.named_scope(f"L{l}_saattn"):
                Y = attention("sa", l, kt_sa, vt_sa, qt, sa_bias, masked=True,
                              probes=(T.get("probes") if l == 0 else None))
            with nc.named_scope(f"L{l}_saop"):
                pr = out_proj_residual("sa", l, Y)
            if l == 0 and T.get("probes") is not None:
                P = T["probes"]
                nc.sync.dma_start(out=P["p_kt"].rearrange("dc p j -> p dc j"), in_=kt_sa)
                nc.sync.dma_start(out=P["p_vt"].rearrange("jc p d -> p jc d"), in_=vt_sa)
                nc.sync.dma_start(out=P["p_qt"].rearrange("dc p i -> p dc i"), in_=qt)
                nc.sync.dma_start(out=P["p_y"].rearrange("dc p i -> p dc i"), in_=Y)
                nc.sync.dma_start(out=P["p_pr"].rearrange("dc p i -> p dc i"), in_=pr)
            with nc.named_scope(f"L{l}_ln1"):
                x_new, xb_new = layer_norm(l, 0, pr, lg_t, lb_t)
            x_cur[0], xb_cur[0] = x_new, xb_new

            # --- cross-attention
            with nc.named_scope(f"L{l}_caq"):
                qt = q_proj("ca", l, ca_bias)
            with nc.named_scope(f"L{l}_caattn"):
                Y = attention("ca", l, kt_ca, vt_ca, qt, ca_bias, masked=False)
            with nc.named_scope(f"L{l}_caop"):
                pr = out_proj_residual("ca", l, Y)
            with nc.named_scope(f"L{l}_ln2"):
                x_new, xb_new = layer_norm(l, 1, pr, lg_t, lb_t)
            x_cur[0], xb_cur[0] = x_new, xb_new

            # --- FFN
            b1t = sm.tile([128, NFF], F32, tag="b1t")
            nc.sync.dma_start(out=b1t,
                              in_=T["b1"][l].rearrange("(fc p) -> p fc", p=128))
            b2t = sm.tile([128, NCH], F32, tag="ob")
            nc.sync.dma_start(out=b2t,
                              in_=T["b2"][l].rearrange("(cc p) -> p cc", p=128))
            hf = hp.tile([128, NFF, NI], BF16, tag="hf")
            sc_ffn1 = nc.named_scope(f"L{l}_ffn1"); sc_ffn1.__enter__()
            for fc in range(NFF):
                w1c = wp.tile([128, NCH, 128], BF16, tag="wA")
                nc.sync.dma_start(out=w1c, in_=T["w1_t"][l, fc])
                acc = ps.tile([128, NI], F32, tag="mm")
                for cc in range(NCH):
                    nc.tensor.matmul(acc[:], w1c[:, cc], xb_cur[0][:, cc],
                                     start=(cc == 0), stop=(cc == NCH - 1))
                nc.scalar.activation(hf[:, fc], acc[:], AF.Relu,
                                     bias=b1t[:, fc:fc + 1])
            sc_ffn1.__exit__(None, None, None)
            pr = pre.tile([128, NCH, NI], F32, tag="preln")
            sc_ffn2 = nc.named_scope(f"L{l}_ffn2"); sc_ffn2.__enter__()
            for cc in range(NCH):
                w2c = wb.tile([128, NFF, 128], BF16, tag="wB")
                nc.sync.dma_start(out=w2c, in_=T["w2_t"][l, cc])
                acc = ps.tile([128, NI], F32, tag="mm")
                for fc in range(NFF):
                    nc.tensor.matmul(acc[:], w2c[:, fc], hf[:, fc],
                                     start=(fc == 0), stop=(fc == NFF - 1))
                tmp = sm.tile([128, NI], F32, tag="tY")
                nc.vector.tensor_scalar(tmp[:], acc[:], b2t[:, cc:cc + 1], None, ALU.add)
                nc.vector.tensor_add(pr[:, cc], tmp[:], x_cur[0][:, cc])
            sc_ffn2.__exit__(None, None, None)
            with nc.named_scope(f"L{l}_ln3"):
                x_new, xb_new = layer_norm(l, 2, pr, lg_t, lb_t)
            x_cur[0], xb_cur[0] = x_new, xb_new

            if dbg is not None:
                nc.sync.dma_start(
                    out=dbg[l].rearrange("(cc p) i -> p cc i", p=128),
                    in_=x_cur[0])

        # --- final projection (weight-tied audio embedding)
        for vc in range(NVC):
            wec = wp.tile([128, NCH, 128], BF16, tag="wA")
            nc.sync.dma_start(out=wec, in_=T["embt"][vc])
            acc = ps.tile([128, NI], F32, tag="mm")
            for cc in range(NCH):
                nc.tensor.matmul(acc[:], wec[:, cc], xb_cur[0][:, cc],
                                 start=(cc == 0), stop=(cc == NCH - 1))
            outt = sm.tile([128, NI], F32, tag="outt")
            nc.vector.tensor_copy(outt[:], acc[:])
            nc.sync.dma_start(out=T["logits_t"][vc], in_=outt[:])


# ----------------------------------------------------------------------------
# public entry
# ----------------------------------------------------------------------------

_PROGRAM_CACHE = {}


def _get_program(n_layers=NL, debug_layers=False, probe=False, trivial_ln=False):
    key = (n_layers, debug_layers, probe, trivial_ln)
    if key not in _PROGRAM_CACHE:
        _PROGRAM_CACHE[key] = build_program(n_layers, debug_layers, probe, trivial_ln)
    return _PROGRAM_CACHE[key]


def _is_trivial_ln(inputs):
    return all(
        np.all(np.asarray(inputs[f"ln{i}_g"]) == 1.0)
        and np.all(np.asarray(inputs[f"ln{i}_b"]) == 0.0)
        for i in (1, 2, 3))


def make_in_maps(inputs, n_layers=NL):
    packed, m01 = host_pack_and_mask(inputs)
    w = host_prep_weights(inputs)
    NI = L // 2
    wmaps = {
        "sa_qk_t": w["sa_qk_t"][:n_layers], "sa_v_t": w["sa_v_t"][:n_layers],
        "sa_qkv_b": w["sa_qkv_b"][:n_layers],
        "sa_o_t": w["sa_o_t"][:n_layers], "sa_o_b": w["sa_o_b"][:n_layers],
        "ca_qk_t": w["ca_qk_t"][:n_layers], "ca_v_t": w["ca_v_t"][:n_layers],
        "ca_qkv_b": w["ca_qkv_b"][:n_layers],
        "ca_o_t": w["ca_o_t"][:n_layers], "ca_o_b": w["ca_o_b"][:n_layers],
        "w1_t": w["w1_t"][:n_layers], "b1": w["b1"][:n_layers],
        "w2_t": w["w2_t"][:n_layers], "b2": w["b2"][:n_layers],
        "ln_g": w["ln_g"][:n_layers], "ln_b": w["ln_b"][:n_layers],
        "embt": w["embt"],
    }
    pm = lambda x, pchunks: np.ascontiguousarray(
        np.asarray(x).reshape(pchunks, 128, -1).transpose(1, 0, 2))
    in_maps = []
    for core in range(N_CORES):
        b, half = core // 2, core % 2
        isl = slice(half * NI, (half + 1) * NI)
        xt = pm(packed[b].T[:, isl].astype(np.float32), NCH)
        memt_full = packed[b].T  # [C, L]
        if half == 0:
            memt = memt_full
        else:
            # own half first so the SPMD own-slice is always cols [0:NI]
            memt = np.concatenate([memt_full[:, NI:], memt_full[:, :NI]], axis=1)
        m01t = pm(np.ascontiguousarray(m01[b].T[:, isl]), NJC)  # [128, NJC, NI]
        in_maps.append({
            "xt": np.ascontiguousarray(xt),
            "memt": _bf(pm(memt, NCH)),
            "m01t": _bf(m01t),
            **wmaps,
        })
    return in_maps


def assemble_output(results):
    NI = L // 2
    out = np.zeros((B, L, CODEC), np.float32)
    for core in range(N_CORES):
        b, half = core // 2, core % 2
        lt = results[core]["logits_t"].reshape(CODEC, NI)
        out[b, half * NI:(half + 1) * NI, :] = lt.T
    return out


def kernel(**inputs):
    nc = _get_program(NL, trivial_ln=_is_trivial_ln(inputs))
    in_maps = make_in_maps(inputs, NL)
    r = run_bass_kernel_spmd(nc, in_maps, list(range(N_CORES)))
    return assemble_output(r.results)


# revision 8
# speedup vs baseline: 1.0097x; 1.0097x over previous
"""Trainium2 Bass kernel for nn_AutoRegressive (6-layer transformer decoder).

Sharding: 4 sample-pairs x 2-way sequence split over 8 NeuronCores.
Core c handles sample c//2, query-token half c%2 (256 tokens).
Per layer, each core computes K/V for its own token half; a pairwise
AllGather assembles full-length K/V. Cross-attention K/V come from the
(constant) packed memory the same way. FFN/LN/projections are per-token.

Device layout: activations transposed [C on partitions (8 chunks of 128),
tokens on free dim]. Matmuls in bf16 with fp32 PSUM accumulation.
Softmax without max-subtraction (scores are small), multiplicative 0/1
mask after exp, denominators via ones-matmuls, normalization via
gpsimd partition_broadcast.

All weights are pre-permuted on the host into partition-major tiles
([128 partitions, free...]) so every weight DMA is a linear copy with
multi-KB per-partition lines (256B-packet DMA was the v1 bottleneck).
"""
import numpy as np
import ml_dtypes

import concourse.bass as bass
import concourse.tile as tile
from concourse import bacc, mybir
from concourse.bass_utils import run_bass_kernel_spmd

F32 = mybir.dt.float32
BF16 = mybir.dt.bfloat16
AF = mybir.ActivationFunctionType
ALU = mybir.AluOpType

B, LT, LA = 4, 64, 448
C, H, DFF, NL = 1024, 16, 4096, 6
VOCAB, CODEC = 256, 1024
L = LT + LA          # 512
DH = C // H          # 64
NCH = C // 128       # 8 feature chunks
NJC = L // 128       # 4 key chunks
NFF = DFF // 128     # 32
NVC = CODEC // 128   # 8
EPS = 1e-5

N_CORES = 8
REPLICA_GROUPS = [[0, 1], [2, 3], [4, 5], [6, 7]]


# ----------------------------------------------------------------------------
# host-side prep
# ----------------------------------------------------------------------------

def _pe_np(length, dim):
    pos = np.arange(length, dtype=np.float32)[:, None]
    div = np.exp(np.arange(0, dim, 2, dtype=np.float32) * (-np.log(10000.0) / dim))
    ang = pos * div
    out = np.zeros((length, dim), np.float32)
    out[:, 0::2] = np.sin(ang)
    out[:, 1::2] = np.cos(ang)
    return out


def host_pack_and_mask(inputs):
    text = np.asarray(inputs["text"]).astype(np.int64)
    audio = np.asarray(inputs["audio"]).astype(np.int64)
    tl = np.asarray(inputs["text_len"]).astype(np.int64)
    al = np.asarray(inputs["audio_len"]).astype(np.int64)
    text_emb = np.asarray(inputs["text_emb"], dtype=np.float32)
    audio_emb = np.asarray(inputs["audio_emb"], dtype=np.float32)

    text_e = text_emb[text] + _pe_np(LT, C)[None]      # [B, LT, C]
    audio_e = audio_emb[audio] + _pe_np(LA, C)[None]   # [B, LA, C]

    packed = np.zeros((B, L, C), np.float32)
    m01 = np.zeros((B, L, L), np.float32)
    i = np.arange(L)[:, None]
    j = np.arange(L)[None, :]
    for b in range(B):
        t, a = int(tl[b]), int(al[b])
        packed[b, :t] = text_e[b, :t]
        packed[b, t:t + a] = audio_e[b, :a]
        il = t + a
        masked = (j > i) & (j >= t) & (i < il) & (j < il)
        m01[b] = np.where(masked, 0.0, 1.0)
    return packed, m01


def _bf(x):
    return np.ascontiguousarray(x).astype(ml_dtypes.bfloat16)


def _pmajor(x, pchunks, tile_w):
    """[R=pchunks*128, W] -> tiles [W//tile_w, 128, pchunks, tile_w] p-major."""
    R, W = x.shape
    assert R == pchunks * 128 and W % tile_w == 0
    return np.ascontiguousarray(
        x.reshape(pchunks, 128, W // tile_w, tile_w).transpose(2, 1, 0, 3))


def host_prep_weights(inputs):
    """Transpose weights to lhsT/rhs conventions, fold the attention scale
    into q, cast to bf16, pre-permute into partition-major DMA tiles."""
    g = lambda n: np.asarray(inputs[n], dtype=np.float32)
    w = {}
    scale = DH ** -0.5
    for pfx in ("sa", "ca"):
        qkv_w = g(f"{pfx}_qkv_w").copy()        # [NL, 3C, C]
        qkv_b = g(f"{pfx}_qkv_b").copy()        # [NL, 3C]
        qkv_w[:, :C] *= scale
        qkv_b[:, :C] *= scale
        qkv_t = qkv_w.transpose(0, 2, 1)        # [NL, C(in), 3C(out)]
        # q/k tiles: [NL, 16, 128, 8, 128]; v: [NL, 2, 128, 8, 512]
        w[f"{pfx}_qk_t"] = _bf(np.stack(
            [_pmajor(qkv_t[l, :, :2 * C], NCH, 128) for l in range(NL)]))
        w[f"{pfx}_v_t"] = _bf(np.stack(
            [_pmajor(qkv_t[l, :, 2 * C:], NCH, 512) for l in range(NL)]))
        w[f"{pfx}_qkv_b"] = np.ascontiguousarray(qkv_b)
        o_t = g(f"{pfx}_out_w").transpose(0, 2, 1)   # [NL, C(d), C(c)]
        w[f"{pfx}_o_t"] = _bf(np.stack(
            [_pmajor(o_t[l], NCH, 128) for l in range(NL)]))  # [NL, 8, 128, 8, 128]
        w[f"{pfx}_o_b"] = np.ascontiguousarray(g(f"{pfx}_out_b"))
    w1_t = g("lin1_w").transpose(0, 2, 1)       # [NL, C, DFF]
    w["w1_t"] = _bf(np.stack([_pmajor(w1_t[l], NCH, 128) for l in range(NL)]))
    w["b1"] = np.ascontiguousarray(g("lin1_b"))
    w2_t = g("lin2_w").transpose(0, 2, 1)       # [NL, DFF, C]
    w["w2_t"] = _bf(np.stack([_pmajor(w2_t[l], NFF, 128) for l in range(NL)]))
    w["b2"] = np.ascontiguousarray(g("lin2_b"))
    ln_g = np.stack([g("ln1_g"), g("ln2_g"), g("ln3_g")], axis=1)  # [NL, 3, C]
    ln_b = np.stack([g("ln1_b"), g("ln2_b"), g("ln3_b")], axis=1)
    w["ln_g"] = np.ascontiguousarray(ln_g)
    w["ln_b"] = np.ascontiguousarray(ln_b)
    w["embt"] = _bf(_pmajor(g("audio_emb").T, NCH, 128))  # [8, 128, 8, 128]
    return w


# ----------------------------------------------------------------------------
# device program
# ----------------------------------------------------------------------------

def build_program(n_layers=NL, debug_layers=False, probe=False, trivial_ln=False):
    """SPMD program for one core: sample = pair, token half = rank in pair."""
    NI = L // 2  # 256 query tokens per core
    NJO = NI // 128  # own key chunks (2)

    nc = bacc.Bacc(None, target_bir_lowering=False)

    # --- external I/O (all partition-major)
    xt = nc.dram_tensor("xt", [128, NCH, NI], F32, kind="ExternalInput")
    memt = nc.dram_tensor("memt", [128, NCH, L], BF16, kind="ExternalInput")
    m01t = nc.dram_tensor("m01t", [128, NJC, NI], BF16, kind="ExternalInput")
    sa_qk_t = nc.dram_tensor("sa_qk_t", [n_layers, 16, 128, NCH, 128], BF16, kind="ExternalInput")
    sa_v_t = nc.dram_tensor("sa_v_t", [n_layers, 2, 128, NCH, 512], BF16, kind="ExternalInput")
    sa_qkv_b = nc.dram_tensor("sa_qkv_b", [n_layers, 3 * C], F32, kind="ExternalInput")
    sa_o_t = nc.dram_tensor("sa_o_t", [n_layers, NCH, 128, NCH, 128], BF16, kind="ExternalInput")
    sa_o_b = nc.dram_tensor("sa_o_b", [n_layers, C], F32, kind="ExternalInput")
    ca_qk_t = nc.dram_tensor("ca_qk_t", [n_layers, 16, 128, NCH, 128], BF16, kind="ExternalInput")
    ca_v_t = nc.dram_tensor("ca_v_t", [n_layers, 2, 128, NCH, 512], BF16, kind="ExternalInput")
    ca_qkv_b = nc.dram_tensor("ca_qkv_b", [n_layers, 3 * C], F32, kind="ExternalInput")
    ca_o_t = nc.dram_tensor("ca_o_t", [n_layers, NCH, 128, NCH, 128], BF16, kind="ExternalInput")
    ca_o_b = nc.dram_tensor("ca_o_b", [n_layers, C], F32, kind="ExternalInput")
    w1_t = nc.dram_tensor("w1_t", [n_layers, NFF, 128, NCH, 128], BF16, kind="ExternalInput")
    b1 = nc.dram_tensor("b1", [n_layers, DFF], F32, kind="ExternalInput")
    w2_t = nc.dram_tensor("w2_t", [n_layers, NCH, 128, NFF, 128], BF16, kind="ExternalInput")
    b2 = nc.dram_tensor("b2", [n_layers, C], F32, kind="ExternalInput")
    ln_g = nc.dram_tensor("ln_g", [n_layers, 3, C], F32, kind="ExternalInput")
    ln_b = nc.dram_tensor("ln_b", [n_layers, 3, C], F32, kind="ExternalInput")
    embt = nc.dram_tensor("embt", [NVC, 128, NCH, 128], BF16, kind="ExternalInput")
    logits_t = nc.dram_tensor("logits_t", [NVC, 128, NI], F32, kind="ExternalOutput")
    dbg = None
    if debug_layers:
        dbg = nc.dram_tensor("dbg", [n_layers, C, NI], F32, kind="ExternalOutput")
    probes = None
    if probe:
        probes = {
            "p_kt": nc.dram_tensor("p_kt", [NCH, 128, L], BF16, kind="ExternalOutput"),
            "p_vt": nc.dram_tensor("p_vt", [NJC, 128, C], BF16, kind="ExternalOutput"),
            "p_qt": nc.dram_tensor("p_qt", [NCH, 128, NI], BF16, kind="ExternalOutput"),
            "p_e": nc.dram_tensor("p_e", [NJC, 128, 8, NI], BF16, kind="ExternalOutput"),
            "p_y": nc.dram_tensor("p_y", [NCH, 128, NI], BF16, kind="ExternalOutput"),
            "p_pr": nc.dram_tensor("p_pr", [NCH, 128, NI], F32, kind="ExternalOutput"),
        }

    # --- internal dram for collectives (one pair per layer per attn kind)
    KVSZ = C * NI  # elements in each of kT_own / v_own packs
    cc_in = {}
    cc_out = {}
    for l in range(n_layers):
        for kind in ("sa", "ca"):
            cc_in[(kind, l)] = nc.dram_tensor(f"{kind}_cci_{l}", [2 * KVSZ], BF16)
            cc_out[(kind, l)] = nc.dram_tensor(f"{kind}_cco_{l}", [2, 2 * KVSZ], BF16)

    with tile.TileContext(nc) as tc:
        _build_body(nc, tc, locals(), n_layers, NI, NJO, dbg, trivial_ln)
    nc.finalize()
    return nc


def _build_body(nc, tc, T, n_layers, NI, NJO, dbg, trivial_ln=False):
    from contextlib import ExitStack
    ctx = ExitStack()
    with ctx:
        const = ctx.enter_context(tc.tile_pool(name="const", bufs=1))
        xpool = ctx.enter_context(tc.tile_pool(name="xpool", bufs=2))
        pre = ctx.enter_context(tc.tile_pool(name="pre", bufs=1))
        kvp = ctx.enter_context(tc.tile_pool(name="kvp", bufs=1))
        kvo = ctx.enter_context(tc.tile_pool(name="kvo", bufs=2))
        ep = ctx.enter_context(tc.tile_pool(name="ep", bufs=1))
        yp = ctx.enter_context(tc.tile_pool(name="yp", bufs=2))
        hp = ctx.enter_context(tc.tile_pool(name="hp", bufs=1))
        wp = ctx.enter_context(tc.tile_pool(name="wp", bufs=6))
        wv = ctx.enter_context(tc.tile_pool(name="wv", bufs=1))
        wb = ctx.enter_context(tc.tile_pool(name="wb", bufs=2))
        sm = ctx.enter_context(tc.tile_pool(name="sm", bufs=2))
        lnp = ctx.enter_context(tc.tile_pool(name="lnp", bufs=1))
        ps = ctx.enter_context(tc.tile_pool(name="ps", bufs=4, space="PSUM"))
        psd = ctx.enter_context(tc.tile_pool(name="psd", bufs=2, space="PSUM"))
        psl = ctx.enter_context(tc.tile_pool(name="psl", bufs=2, space="PSUM"))

        # ---- constants
        memt_t = const.tile([128, NCH, L], BF16)
        nc.sync.dma_start(out=memt_t, in_=T["memt"][:])
        m01_t = const.tile([128, NJC, NI], BF16)
        nc.sync.dma_start(out=m01_t, in_=T["m01t"][:])
        ones_bf = const.tile([128, 1], BF16)
        nc.vector.memset(ones_bf, 1.0)
        ones_f = const.tile([128, 1], F32)
        nc.vector.memset(ones_f, 1.0)
        eps_t = const.tile([1, 1], F32)
        nc.vector.memset(eps_t, EPS)

        # ---- layer 0 activations
        x_f = xpool.tile([128, NCH, NI], F32, tag="x")
        nc.sync.dma_start(out=x_f, in_=T["xt"][:])
        xb = xpool.tile([128, NCH, NI], BF16, tag="xb")
        nc.vector.tensor_copy(xb, x_f)

        memt_bf = memt_t  # alias

        def _layer_bias(kind, l):
            bt = sm.tile([128, 24], F32, tag=f"b_{kind}")
            nc.sync.dma_start(
                out=bt, in_=T[f"{kind}_qkv_b"][l].rearrange("(dc p) -> p dc", p=128))
            return bt

        def kv_own_and_gather(kind, l, src_bf, src_islice):
            """Compute own-half K/V, pack, AllGather; return full
            kT [128,NCH,L] bf16 and v [128,NJC,C] bf16 (global token order)."""
            qk_t = T[f"{kind}_qk_t"]
            bias_tile = _layer_bias(kind, l)
            k_own = kvo.tile([128, NCH, NI], BF16, tag="k_own")
            for dc in range(NCH):
                wk = wp.tile([128, NCH, 128], BF16, tag="wA")
                nc.sync.dma_start(out=wk, in_=qk_t[l, 8 + dc])
                acc = ps.tile([128, NI], F32, tag="mm")
                for cc in range(NCH):
                    nc.tensor.matmul(acc[:], wk[:, cc], src_bf[:, cc, src_islice],
                                     start=(cc == 0), stop=(cc == NCH - 1))
                nc.vector.tensor_scalar(k_own[:, dc], acc[:],
                                        bias_tile[:, 8 + dc:9 + dc], None, ALU.add)
            # v_own [128, NJO(j), C]  (v-bias applied post-normalize)
            v_own = kvo.tile([128, NJO, C], BF16, tag="v_own")
            for ds in range(2):
                wvt = wv.tile([128, NCH, 512], BF16, tag="wV")
                nc.sync.dma_start(out=wvt, in_=T[f"{kind}_v_t"][l, ds])
                for jo in range(NJO):
                    acc = ps.tile([128, 512], F32, tag="mm")
                    for cc in range(NCH):
                        lhs = src_bf[:, cc, src_islice][:, jo * 128:(jo + 1) * 128]
                        nc.tensor.matmul(acc[:], lhs, wvt[:, cc],
                                         start=(cc == 0), stop=(cc == NCH - 1))
                    nc.vector.tensor_copy(v_own[:, jo, ds * 512:(ds + 1) * 512], acc[:])
            # pack to dram: [kT_own | v_own], both partition-major
            cci = T["cc_in"][(kind, l)]
            cco = T["cc_out"][(kind, l)]
            KVSZ = C * NI
            nc.sync.dma_start(
                out=cci[0:KVSZ].rearrange("(p dc i) -> p dc i", p=128, i=NI),
                in_=k_own)
            nc.sync.dma_start(
                out=cci[KVSZ:2 * KVSZ].rearrange("(p jo d) -> p jo d", p=128, d=C),
                in_=v_own)
            nc.gpsimd.collective_compute(
                "AllGather", ALU.bypass, ins=[cci[:]], outs=[cco[:]],
                replica_groups=REPLICA_GROUPS)
            kt = kvp.tile([128, NCH, L], BF16, tag=f"kt_{kind}")
            vt = kvp.tile([128, NJC, C], BF16, tag=f"vt_{kind}")
            for blk in range(2):
                nc.sync.dma_start(
                    out=kt[:, :, blk * NI:(blk + 1) * NI],
                    in_=cco[blk, 0:KVSZ].rearrange("(p dc i) -> p dc i", p=128, i=NI))
                nc.sync.dma_start(
                    out=vt[:, blk * NJO:(blk + 1) * NJO, :],
                    in_=cco[blk, KVSZ:2 * KVSZ].rearrange("(p jo d) -> p jo d", p=128, d=C))
            return kt, vt

        def q_proj(kind, l, bias_tile):
            qt = yp.tile([128, NCH, NI], BF16, tag="qt")
            qk_t = T[f"{kind}_qk_t"]
            for dc in range(NCH):
                wq = wp.tile([128, NCH, 128], BF16, tag="wA")
                nc.sync.dma_start(out=wq, in_=qk_t[l, dc])
                acc = ps.tile([128, NI], F32, tag="mm")
                for cc in range(NCH):
                    nc.tensor.matmul(acc[:], wq[:, cc], xb_cur[0][:, cc],
                                     start=(cc == 0), stop=(cc == NCH - 1))
                nc.vector.tensor_scalar(qt[:, dc], acc[:],
                                        bias_tile[:, dc:dc + 1], None, ALU.add)
            return qt

        def attention(kind, l, kt, vt, qt, bias_tile, masked, probes=None):
            """-> Y bf16 [128, NCH, NI] (normalized, v-bias added)."""
            Y = yp.tile([128, NCH, NI], BF16, tag="Y")
            for g in range(2):           # head groups of 8
                et = ep.tile([128, NJC, 8, NI], BF16, tag="ee")
                if probes is not None and g == 0:
                    et_probe = et
                for hh in range(8):
                    h = g * 8 + hh
                    hc, hs = h // 2, (h % 2) * 64
                    for jc in range(NJC):
                        sc = ps.tile([128, NI], F32, tag="mm")
                        nc.tensor.matmul(
                            sc[:],
                            kt[hs:hs + 64, hc, jc * 128:(jc + 1) * 128],
                            qt[hs:hs + 64, hc, :], start=True, stop=True)
                        nc.scalar.activation(et[:, jc, hh], sc[:], AF.Exp)
                        if masked:
                            nc.vector.tensor_mul(et[:, jc, hh], et[:, jc, hh],
                                                 m01_t[:, jc])
                for hp2 in range(4):     # head pairs within group
                    dc = g * 4 + hp2
                    ypv = ps.tile([128, NI], F32, tag="mm")
                    dA = psd.tile([1, NI], F32, tag="den")
                    dB = psd.tile([1, NI], F32, tag="den")
                    for jc in range(NJC):
                        nc.tensor.matmul(
                            ypv[0:64, :], vt[:, jc, dc * 128:dc * 128 + 64],
                            et[:, jc, 2 * hp2], start=(jc == 0), stop=(jc == NJC - 1),
                            tile_position=(0, 0))
                        nc.tensor.matmul(
                            ypv[64:128, :], vt[:, jc, dc * 128 + 64:dc * 128 + 128],
                            et[:, jc, 2 * hp2 + 1], start=(jc == 0), stop=(jc == NJC - 1),
                            tile_position=(0, 64))
                        nc.tensor.matmul(dA[:], ones_bf[:], et[:, jc, 2 * hp2],
                                         start=(jc == 0), stop=(jc == NJC - 1))
                        nc.tensor.matmul(dB[:], ones_bf[:], et[:, jc, 2 * hp2 + 1],
                                         start=(jc == 0), stop=(jc == NJC - 1))
                    rec = sm.tile([1, 2, NI], F32, tag="rec")
                    nc.vector.reciprocal(rec[:, 0], dA[:])
                    nc.vector.reciprocal(rec[:, 1], dB[:])
                    recbA = sm.tile([128, NI], F32, tag="recbA")
                    recbB = sm.tile([128, NI], F32, tag="recbB")
                    nc.gpsimd.partition_broadcast(recbA[:], rec[:, 0])
                    nc.gpsimd.partition_broadcast(recbB[:], rec[:, 1])
                    tY = sm.tile([128, NI], F32, tag="tY")
                    nc.vector.tensor_mul(tY[0:64, :], ypv[0:64, :], recbA[0:64, :])
                    nc.vector.tensor_mul(tY[64:128, :], ypv[64:128, :], recbB[64:128, :])
                    nc.scalar.activation(Y[:, dc], tY[:], AF.Identity,
                                         bias=bias_tile[:, 16 + dc:17 + dc])
                if probes is not None and g == 0:
                    nc.sync.dma_start(
                        out=probes["p_e"].rearrange("jc p h i -> p jc h i"),
                        in_=et_probe)
            return Y

        def out_proj_residual(kind, l, Y):
            """attn out-proj + bias + residual -> preln f32 [128, NCH, NI]."""
            ot = T[f"{kind}_o_t"]
            obt = sm.tile([128, NCH], F32, tag="ob")
            nc.sync.dma_start(out=obt,
                              in_=T[f"{kind}_o_b"][l].rearrange("(cc p) -> p cc", p=128))
            pr = pre.tile([128, NCH, NI], F32, tag="preln")
            for cc in range(NCH):
                wo = wp.tile([128, NCH, 128], BF16, tag="wA")
                nc.sync.dma_start(out=wo, in_=ot[l, cc])
                acc = ps.tile([128, NI], F32, tag="mm")
                for dc in range(NCH):
                    nc.tensor.matmul(acc[:], wo[:, dc], Y[:, dc],
                                     start=(dc == 0), stop=(dc == NCH - 1))
                tmp = sm.tile([128, NI], F32, tag="tY")
                nc.vector.tensor_scalar(tmp[:], acc[:], obt[:, cc:cc + 1], None, ALU.add)
                nc.vector.tensor_add(pr[:, cc], tmp[:], x_cur[0][:, cc])
            return pr

        def layer_norm(l, idx, pr, lg_t, lb_t):
            """LN over partitions-dim features; returns (x_f32, xb_bf16)."""
            s_ps = psl.tile([1, NI], F32, tag="lns")
            q_ps = psl.tile([1, NI], F32, tag="lns")
            for cc in range(NCH):
                xsq = sm.tile([128, NI], F32, tag="xsq")
                nc.scalar.activation(xsq[:], pr[:, cc], AF.Square)
                nc.tensor.matmul(s_ps[:], ones_f[:], pr[:, cc],
                                 start=(cc == 0), stop=(cc == NCH - 1))
                nc.tensor.matmul(q_ps[:], ones_f[:], xsq[:],
                                 start=(cc == 0), stop=(cc == NCH - 1))
            mu = lnp.tile([1, NI], F32, tag="mu")
            nc.scalar.activation(mu[:], s_ps[:], AF.Identity, scale=1.0 / C)
            var = lnp.tile([1, NI], F32, tag="var")
            nc.scalar.activation(var[:], q_ps[:], AF.Identity, scale=1.0 / C)
            musq = lnp.tile([1, NI], F32, tag="musq")
            nc.vector.tensor_mul(musq[:], mu[:], mu[:])
            nc.vector.tensor_sub(var[:], var[:], musq[:])
            sd = lnp.tile([1, NI], F32, tag="sd")
            nc.scalar.activation(sd[:], var[:], AF.Sqrt, bias=eps_t[:])
            rmur = lnp.tile([1, 2, NI], F32, tag="rmur")
            nc.vector.reciprocal(rmur[:, 0], sd[:])
            nc.vector.tensor_mul(rmur[:, 1], mu[:], rmur[:, 0])
            rb = lnp.tile([128, 2, NI], F32, tag="rb")
            nc.gpsimd.partition_broadcast(rb[:], rmur[:])
            x_new = xpool.tile([128, NCH, NI], F32, tag="x")
            xb_new = xpool.tile([128, NCH, NI], BF16, tag="xb")
            for cc in range(NCH):
                t1 = sm.tile([128, NI], F32, tag="t1")
                nc.vector.tensor_mul(t1[:], pr[:, cc], rb[:, 0])
                if trivial_ln:
                    nc.vector.tensor_sub(x_new[:, cc], t1[:], rb[:, 1])
                else:
                    nc.vector.tensor_sub(t1[:], t1[:], rb[:, 1])
                    nc.scalar.activation(x_new[:, cc], t1[:], AF.Identity,
                                         bias=lb_t[:, idx, cc:cc + 1],
                                         scale=lg_t[:, idx, cc:cc + 1])
                nc.vector.tensor_copy(xb_new[:, cc], x_new[:, cc])
            return x_new, xb_new

        # mutable current-activation refs
        x_cur = [x_f]
        xb_cur = [xb]

        for l in range(n_layers):
            lg_t = sm.tile([128, 3, NCH], F32, tag="lg")
            lb_t = sm.tile([128, 3, NCH], F32, tag="lb")
            nc.sync.dma_start(out=lg_t,
                              in_=T["ln_g"][l].rearrange("three (cc p) -> p three cc", p=128))
            nc.sync.dma_start(out=lb_t,
                              in_=T["ln_b"][l].rearrange("three (cc p) -> p three cc", p=128))

            # SA K/V + AllGather kick first (x just became available)
            sa_bias = _layer_bias("sa", l)
            kt_sa, vt_sa = kv_own_and_gather("sa", l, xb_cur[0], slice(0, NI))

            # --- self-attention
            qt = q_proj("sa", l, sa_bias)
            # CA K/V from constant memory (memt is pre-rolled per core: own half
            # first, so the SPMD own-slice is always cols [0:NI]; the AllGather
            # restores global order). Emitted here as a lower-priority gap
            # filler for the SA AllGather wait and the LN1 serial chain.
            ca_bias = _layer_bias("ca", l)
            kt_ca, vt_ca = kv_own_and_gather("ca", l, memt_bf, slice(0, NI))

            Y = attention("sa", l, kt_sa, vt_sa, qt, sa_bias, masked=True,
                          probes=(T.get("probes") if l == 0 else None))
            pr = out_proj_residual("sa", l, Y)
            if l == 0 and T.get("probes") is not None:
                P = T["probes"]
                nc.sync.dma_start(out=P["p_kt"].rearrange("dc p j -> p dc j"), in_=kt_sa)
                nc.sync.dma_start(out=P["p_vt"].rearrange("jc p d -> p jc d"), in_=vt_sa)
                nc.sync.dma_start(out=P["p_qt"].rearrange("dc p i -> p dc i"), in_=qt)
                nc.sync.dma_start(out=P["p_y"].rearrange("dc p i -> p dc i"), in_=Y)
                nc.sync.dma_start(out=P["p_pr"].rearrange("dc p i -> p dc i"), in_=pr)
            x_new, xb_new = layer_norm(l, 0, pr, lg_t, lb_t)
            x_cur[0], xb_cur[0] = x_new, xb_new

            # --- cross-attention
            qt = q_proj("ca", l, ca_bias)
            Y = attention("ca", l, kt_ca, vt_ca, qt, ca_bias, masked=False)
            pr = out_proj_residual("ca", l, Y)
            x_new, xb_new = layer_norm(l, 1, pr, lg_t, lb_t)
            x_cur[0], xb_cur[0] = x_new, xb_new

            # --- FFN
            b1t = sm.tile([128, NFF], F32, tag="b1t")
            nc.sync.dma_start(out=b1t,
                              in_=T["b1"][l].rearrange("(fc p) -> p fc", p=128))
            b2t = sm.tile([128, NCH], F32, tag="ob")
            nc.sync.dma_start(out=b2t,
                              in_=T["b2"][l].rearrange("(cc p) -> p cc", p=128))
            hf = hp.tile([128, NFF, NI], BF16, tag="hf")
            for fc in range(NFF):
                w1c = wp.tile([128, NCH, 128], BF16, tag="wA")
                nc.sync.dma_start(out=w1c, in_=T["w1_t"][l, fc])
                acc = ps.tile([128, NI], F32, tag="mm")
                for cc in range(NCH):
                    nc.tensor.matmul(acc[:], w1c[:, cc], xb_cur[0][:, cc],
                                     start=(cc == 0), stop=(cc == NCH - 1))
                nc.scalar.activation(hf[:, fc], acc[:], AF.Relu,
                                     bias=b1t[:, fc:fc + 1])
            pr = pre.tile([128, NCH, NI], F32, tag="preln")
            for cc in range(NCH):
                w2c = wb.tile([128, NFF, 128], BF16, tag="wB")
                nc.sync.dma_start(out=w2c, in_=T["w2_t"][l, cc])
                acc = ps.tile([128, NI], F32, tag="mm")
                for fc in range(NFF):
                    nc.tensor.matmul(acc[:], w2c[:, fc], hf[:, fc],
                                     start=(fc == 0), stop=(fc == NFF - 1))
                tmp = sm.tile([128, NI], F32, tag="tY")
                nc.vector.tensor_scalar(tmp[:], acc[:], b2t[:, cc:cc + 1], None, ALU.add)
                nc.vector.tensor_add(pr[:, cc], tmp[:], x_cur[0][:, cc])
            x_new, xb_new = layer_norm(l, 2, pr, lg_t, lb_t)
            x_cur[0], xb_cur[0] = x_new, xb_new

            if dbg is not None:
                nc.sync.dma_start(
                    out=dbg[l].rearrange("(cc p) i -> p cc i", p=128),
                    in_=x_cur[0])

        # --- final projection (weight-tied audio embedding)
        for vc in range(NVC):
            wec = wp.tile([128, NCH, 128], BF16, tag="wA")
            nc.sync.dma_start(out=wec, in_=T["embt"][vc])
            acc = ps.tile([128, NI], F32, tag="mm")
            for cc in range(NCH):
                nc.tensor.matmul(acc[:], wec[:, cc], xb_cur[0][:, cc],
                                 start=(cc == 0), stop=(cc == NCH - 1))
            outt = sm.tile([128, NI], F32, tag="outt")
            nc.vector.tensor_copy(outt[:], acc[:])
            nc.sync.dma_start(out=T["logits_t"][vc], in_=outt[:])


# ----------------------------------------------------------------------------
# public entry
# ----------------------------------------------------------------------------

_PROGRAM_CACHE = {}


def _get_program(n_layers=NL, debug_layers=False, probe=False, trivial_ln=False):
    key = (n_layers, debug_layers, probe, trivial_ln)
    if key not in _PROGRAM_CACHE:
        _PROGRAM_CACHE[key] = build_program(n_layers, debug_layers, probe, trivial_ln)
    return _PROGRAM_CACHE[key]


def _is_trivial_ln(inputs):
    return all(
        np.all(np.asarray(inputs[f"ln{i}_g"]) == 1.0)
        and np.all(np.asarray(inputs[f"ln{i}_b"]) == 0.0)
        for i in (1, 2, 3))


def make_in_maps(inputs, n_layers=NL):
    packed, m01 = host_pack_and_mask(inputs)
    w = host_prep_weights(inputs)
    NI = L // 2
    wmaps = {
        "sa_qk_t": w["sa_qk_t"][:n_layers], "sa_v_t": w["sa_v_t"][:n_layers],
        "sa_qkv_b": w["sa_qkv_b"][:n_layers],
        "sa_o_t": w["sa_o_t"][:n_layers], "sa_o_b": w["sa_o_b"][:n_layers],
        "ca_qk_t": w["ca_qk_t"][:n_layers], "ca_v_t": w["ca_v_t"][:n_layers],
        "ca_qkv_b": w["ca_qkv_b"][:n_layers],
        "ca_o_t": w["ca_o_t"][:n_layers], "ca_o_b": w["ca_o_b"][:n_layers],
        "w1_t": w["w1_t"][:n_layers], "b1": w["b1"][:n_layers],
        "w2_t": w["w2_t"][:n_layers], "b2": w["b2"][:n_layers],
        "ln_g": w["ln_g"][:n_layers], "ln_b": w["ln_b"][:n_layers],
        "embt": w["embt"],
    }
    pm = lambda x, pchunks: np.ascontiguousarray(
        np.asarray(x).reshape(pchunks, 128, -1).transpose(1, 0, 2))
    in_maps = []
    for core in range(N_CORES):
        b, half = core // 2, core % 2
        isl = slice(half * NI, (half + 1) * NI)
        xt = pm(packed[b].T[:, isl].astype(np.float32), NCH)
        memt_full = packed[b].T  # [C, L]
        if half == 0:
            memt = memt_full
        else:
            # own half first so the SPMD own-slice is always cols [0:NI]
            memt = np.concatenate([memt_full[:, NI:], memt_full[:, :NI]], axis=1)
        m01t = pm(np.ascontiguousarray(m01[b].T[:, isl]), NJC)  # [128, NJC, NI]
        in_maps.append({
            "xt": np.ascontiguousarray(xt),
            "memt": _bf(pm(memt, NCH)),
            "m01t": _bf(m01t),
            **wmaps,
        })
    return in_maps


def assemble_output(results):
    NI = L // 2
    out = np.zeros((B, L, CODEC), np.float32)
    for core in range(N_CORES):
        b, half = core // 2, core % 2
        lt = results[core]["logits_t"].reshape(CODEC, NI)
        out[b, half * NI:(half + 1) * NI, :] = lt.T
    return out


def kernel(**inputs):
    nc = _get_program(NL, trivial_ln=_is_trivial_ln(inputs))
    in_maps = make_in_maps(inputs, NL)
    r = run_bass_kernel_spmd(nc, in_maps, list(range(N_CORES)))
    return assemble_output(r.results)
